# revision 3
# baseline (speedup 1.0000x reference)
"""Trainium2 Bass kernel for nn_ExcitatoryInhibitoryBlock.

Math reformulation (eliminates the [T_MAX, B, in, out] scatter buffers):
  o_ih[t] = b_ih + sum_d x[t-d] @ Wd_ih[d],  Wd_ih[d][i,o] = |W_ih.T|*mask_ih.T * (delay_ih==d)
  o_hh[t] = b_hh + sum_d (s[t-1-d]*ei) @ Wd_hh[d]
  u  = u*alpha + (1-alpha)*RM*(o_ih + o_hh)
  s  = (u - (B0 + BETA*eta) >= 0);  u -= s*(BETA*eta + B0);  eta = rho*eta + (1-rho)*s

Sharding: data-parallel over batch B=8 -> one batch element per NeuronCore,
no collectives (the recurrence is independent per batch element).

Device layout: neurons n = ob*128 + p (p = SBUF partition, ob in 0..2).
All weights are host-prepacked into [128, ktile, ob, col] K-tile layout so the
tensor engine runs  out[128 o, N] += W_block[128 i, 128 o]^T @ rhs[128 i, N].
"""

import sys

if "/opt/trn_rl_repo" not in sys.path:
    sys.path.insert(0, "/opt/trn_rl_repo")

import numpy as np

T, T_MAX = 12, 4
B = 8
N_PRE, N_NEU = 192, 384
N_EXC = int(N_NEU * 0.8)
RM, DT, B0, BETA = 1.0, 1.0, 0.01, 1.8

NB = N_NEU // 128          # 3 output column-blocks
KIH = 7                    # ceil((T_MAX*N_PRE + 1)/128): 768 rows + bias row -> 896
KHH = T_MAX * NB           # 12 k-tiles of the delay-expanded hh weights

_CACHE = {}
LAST_RESULTS = None


def _prep_weights(W_ih, b_ih, W_hh, b_hh, tau_m, tau_adp,
                  mask_ih, mask_hh, delay_ih, delay_hh):
    f32 = np.float32
    Wm_ih = (np.abs(W_ih.T) * mask_ih.T).astype(f32)       # [192, 384]
    Wm_hh = (np.abs(W_hh.T) * mask_hh.T).astype(f32)       # [384, 384]
    ei = np.concatenate([np.ones(N_EXC, f32), -np.ones(N_NEU - N_EXC, f32)])
    Wm_hh = Wm_hh * ei[:, None]

    # ih: rows (d*192 + i) for d<4, row 768 = fused bias (b_ih + b_hh), pad to 896
    wih_cat = np.zeros((KIH * 128, N_NEU), f32)
    for d in range(T_MAX):
        wih_cat[d * N_PRE:(d + 1) * N_PRE] = Wm_ih * (delay_ih == d)
    wih_cat[T_MAX * N_PRE] = (b_ih + b_hh).astype(f32)
    # -> [128 p, kt, ob, col]
    wih = np.ascontiguousarray(
        wih_cat.reshape(KIH, 128, NB, 128).transpose(1, 0, 2, 3))

    # hh: k-tile (d*3 + kt) holds rows i = kt*128 + p of Wd_hh[d]
    whh_cat = np.zeros((KHH * 128, N_NEU), f32)
    for d in range(T_MAX):
        whh_cat[d * N_NEU:(d + 1) * N_NEU] = Wm_hh * (delay_hh == d)
    whh = np.ascontiguousarray(
        whh_cat.reshape(KHH, 128, NB, 128).transpose(1, 0, 2, 3))

    alpha = np.exp(-np.float32(DT) / tau_m).astype(f32)
    rho = np.exp(-np.float32(DT) / tau_adp).astype(f32)
    consts = np.zeros((128, 4, NB), f32)
    consts[:, 0, :] = alpha.reshape(NB, 128).T
    consts[:, 1, :] = ((np.float32(1.0) - alpha) * np.float32(RM)).reshape(NB, 128).T
    consts[:, 2, :] = rho.reshape(NB, 128).T
    consts[:, 3, :] = (np.float32(1.0) - rho).reshape(NB, 128).T
    return wih, whh, consts


def _prep_x(x):
    """x [T, B, N_PRE] -> per-core xcat [128, KIH, T]; row d*192+i of col t = x[t-d, b, i]."""
    f32 = np.float32
    out = []
    for b in range(B):
        xc = np.zeros((KIH * 128, T), f32)
        for d in range(T_MAX):
            for t in range(d, T):
                xc[d * N_PRE:(d + 1) * N_PRE, t] = x[t - d, b]
        xc[T_MAX * N_PRE, :] = 1.0  # bias row
        out.append(np.ascontiguousarray(xc.reshape(KIH, 128, T).transpose(1, 0, 2)))
    return out


def _build():
    import concourse.bacc as bacc
    import concourse.tile as tile
    from concourse import mybir

    f32 = mybir.dt.float32
    nc = bacc.Bacc("TRN2", target_bir_lowering=False, debug=False)
    xc_d = nc.dram_tensor("xcat", [128, KIH, T], f32, kind="ExternalInput")
    wih_d = nc.dram_tensor("wih", [128, KIH, NB, 128], f32, kind="ExternalInput")
    whh_d = nc.dram_tensor("whh", [128, KHH, NB, 128], f32, kind="ExternalInput")
    cons_d = nc.dram_tensor("consts", [128, 4, NB], f32, kind="ExternalInput")
    sout_d = nc.dram_tensor("sout", [128, T, NB], f32, kind="ExternalOutput")

    with tile.TileContext(nc) as tc:
        with (
            tc.tile_pool(name="singles", bufs=1) as singles,
            tc.tile_pool(name="state", bufs=3) as state,
            tc.tile_pool(name="tmp", bufs=3) as tmp,
            tc.tile_pool(name="psi", bufs=1, space="PSUM") as psi,
            tc.tile_pool(name="psh", bufs=5, space="PSUM") as psh,
        ):
            wih_sb = singles.tile([128, KIH, NB, 128], f32)
            whh_sb = singles.tile([128, KHH, NB, 128], f32)
            xc_sb = singles.tile([128, KIH, T], f32)
            cons_sb = singles.tile([128, 4, NB], f32)
            s_all = singles.tile([128, T, NB], f32)

            nc.sync.dma_start(out=xc_sb[:], in_=xc_d[:])
            nc.sync.dma_start(out=cons_sb[:], in_=cons_d[:])
            nc.sync.dma_start(out=wih_sb[:], in_=wih_d[:])
            # split hh weight dma by delay so early matmuls can start sooner
            for d in range(T_MAX):
                nc.sync.dma_start(out=whh_sb[:, d * NB:(d + 1) * NB],
                                  in_=whh_d[:, d * NB:(d + 1) * NB])

            alpha = cons_sb[:, 0, :]
            afac = cons_sb[:, 1, :]
            rho = cons_sb[:, 2, :]
            rho1 = cons_sb[:, 3, :]

            # ---- input-synapse conv: psum_ih[128 o, ob, t] for all t at once
            psum_ih = psi.tile([128, NB, T], f32)
            for ob in range(NB):
                for kt in range(KIH):
                    nc.tensor.matmul(
                        psum_ih[:, ob, :],
                        wih_sb[:, kt, ob, :],
                        xc_sb[:, kt, :],
                        start=(kt == 0), stop=(kt == KIH - 1),
                    )

            cur_ih = singles.tile([128, NB, T], f32)
            nc.vector.tensor_copy(cur_ih[:], psum_ih[:])

            u_prev = None
            eta_prev = None
            for t in range(T):
                # ---- recurrent synapse matmuls (emit oldest delay first)
                pairs = [(d, kt) for d in range(T_MAX - 1, -1, -1)
                         for kt in range(NB) if t - 1 - d >= 0]
                psum_hh = None
                if pairs:
                    psum_hh = psh.tile([128, NB], f32, tag="psum_hh")
                    for ob in range(NB):
                        for i, (d, kt) in enumerate(pairs):
                            nc.tensor.matmul(
                                psum_hh[:, ob:ob + 1],
                                whh_sb[:, d * NB + kt, ob, :],
                                s_all[:, t - 1 - d, kt:kt + 1],
                                start=(i == 0), stop=(i == len(pairs) - 1),
                            )

                # ---- pointwise neuron update on [128, NB]
                ih_col = cur_ih[:, :, t]
                if psum_hh is not None:
                    i_t = tmp.tile([128, NB], f32, tag="i_t")
                    nc.vector.tensor_add(i_t[:], psum_hh[:], ih_col)
                    i_t = i_t[:]
                else:
                    i_t = ih_col
                ua = tmp.tile([128, NB], f32, tag="ua")
                ia = tmp.tile([128, NB], f32, tag="ia")
                un = state.tile([128, NB], f32, tag="u")
                if u_prev is not None:
                    nc.vector.tensor_mul(ua[:], u_prev[:], alpha)
                nc.vector.tensor_mul(ia[:], i_t, afac)
                if u_prev is not None:
                    nc.vector.tensor_add(un[:], ua[:], ia[:])
                else:
                    nc.vector.tensor_copy(un[:], ia[:])

                vthr = tmp.tile([128, NB], f32, tag="vthr")
                delta = tmp.tile([128, NB], f32, tag="delta")
                if eta_prev is not None:
                    nc.vector.tensor_scalar(
                        out=vthr[:], in0=eta_prev[:], scalar1=float(BETA),
                        scalar2=float(B0), op0=mybir.AluOpType.mult,
                        op1=mybir.AluOpType.add)
                else:
                    nc.vector.memset(vthr[:], float(B0))
                nc.vector.tensor_sub(delta[:], un[:], vthr[:])
                s_t = s_all[:, t, :]
                nc.vector.tensor_scalar(
                    out=s_t, in0=delta[:], scalar1=0.0, scalar2=None,
                    op0=mybir.AluOpType.is_ge)

                # u'' = u' - s*vthr ; eta' = rho*eta + (1-rho)*s
                sv = tmp.tile([128, NB], f32, tag="sv")
                u2 = state.tile([128, NB], f32, tag="u2")
                nc.vector.tensor_mul(sv[:], s_t, vthr[:])
                nc.vector.tensor_sub(u2[:], un[:], sv[:])
                er = tmp.tile([128, NB], f32, tag="er")
                sr = tmp.tile([128, NB], f32, tag="sr")
                etan = state.tile([128, NB], f32, tag="eta")
                nc.vector.tensor_mul(sr[:], s_t, rho1)
                if eta_prev is not None:
                    nc.vector.tensor_mul(er[:], eta_prev[:], rho)
                    nc.vector.tensor_add(etan[:], er[:], sr[:])
                else:
                    nc.vector.tensor_copy(etan[:], sr[:])
                u_prev, eta_prev = u2, etan

            nc.sync.dma_start(out=sout_d[:], in_=s_all[:])

    nc.compile()
    return nc


def kernel(x, W_ih, b_ih, W_hh, b_hh, tau_m, tau_adp,
           mask_ih, mask_hh, delay_ih, delay_hh, _trace=False, _tmpdir=None):
    global LAST_RESULTS
    from concourse.bass_utils import run_bass_kernel_spmd

    if "nc" not in _CACHE:
        _CACHE["nc"] = _build()
    nc = _CACHE["nc"]

    wih, whh, consts = _prep_weights(
        np.asarray(W_ih), np.asarray(b_ih), np.asarray(W_hh), np.asarray(b_hh),
        np.asarray(tau_m), np.asarray(tau_adp), np.asarray(mask_ih),
        np.asarray(mask_hh), np.asarray(delay_ih), np.asarray(delay_hh))
    xcats = _prep_x(np.asarray(x))

    in_maps = [
        {"xcat": xcats[b], "wih": wih, "whh": whh, "consts": consts}
        for b in range(B)
    ]
    res = run_bass_kernel_spmd(nc, in_maps, list(range(B)),
                               trace=_trace, tmpdir=_tmpdir)
    LAST_RESULTS = res
    out = np.empty((T, B, N_NEU), np.float32)
    for b in range(B):
        sb = res.results[b]["sout"]          # [128, T, NB]
        out[:, b, :] = sb.transpose(1, 2, 0).reshape(T, N_NEU)
    return out


# revision 5
# speedup vs baseline: 3.7366x; 3.7366x over previous
"""Trainium2 Bass kernel for nn_ExcitatoryInhibitoryBlock.

Math reformulation (eliminates the [T_MAX, B, in, out] scatter buffers):
  o_ih[t] = b_ih + sum_d x[t-d] @ Wd_ih[d],  Wd_ih[d][i,o] = |W_ih.T|*mask_ih.T * (delay_ih==d)
  o_hh[t] = b_hh + sum_d (s[t-1-d]*ei) @ Wd_hh[d]
  u  = u*alpha + (1-alpha)*RM*(o_ih + o_hh)
  s  = (u - (B0 + BETA*eta) >= 0);  u -= s*(BETA*eta + B0);  eta = rho*eta + (1-rho)*s

Sharding: data-parallel over batch B=8 -> one batch element per NeuronCore,
no collectives (the recurrence is independent per batch element).

Precision scheme: the PE runs fp16 (fp32 matmul is ~4x slower / 2-pass).
(1-alpha)*RM is folded into all synapse weights + bias on the host, then each
folded weight W' is split W' ~= W_hi + 2^-11 * W_lo with W_hi = fp16(W'),
W_lo = fp16((W' - W_hi) * 2^11).  The 2^-11 rides on the rhs: spikes are fed
twice (s and s*2^-11, both exact in fp16), x is split the same way.  Residual
error is ~2^-22 * |W'| with |W'| ~ 5e-3 * |W|, far below fp32 rounding of u.

Device layout: neurons n = ob*128 + p (p = SBUF partition, ob in 0..2).
Weights are host-prepacked [128 p, ktile, ob, col]; the tensor engine runs
out[128 o, N] += W_block[128 i, 128 o]^T @ rhs[128 i, N].

Schedule: PSUM holds i_t*afac for 4 steps per bank (3 chunk tiles).  The ih
conv (T1) initializes them; recurrent blocks accumulate start=False.  Delay-d
contributions for steps [t0, t0+w) batch into one matmul (w <= d, rhs =
consecutive spike columns) emitted in slot t0 so only d=0 blocks sit on the
s[t-1] -> s[t] critical path.  Off-path state ops (eta, vthr, u'', wc) run
while the PE does next-slot prefetch; the critical DVE chain per step is
delta = psum_col + wc ; s = is_ge(delta, 0).
"""

import sys

if "/opt/trn_rl_repo" not in sys.path:
    sys.path.insert(0, "/opt/trn_rl_repo")

import numpy as np

T, T_MAX = 12, 4
B = 8
N_PRE, N_NEU = 192, 384
N_EXC = int(N_NEU * 0.8)
RM, DT, B0, BETA = 1.0, 1.0, 0.01, 1.8

NB = N_NEU // 128          # 3 output column-blocks
KIH = 7                    # ceil((T_MAX*N_PRE + 1)/128): 768 rows + bias row
KHH = T_MAX * NB           # 12 k-tiles of delay-expanded hh weights
NCH = 3                    # psum chunks of 4 steps
CW = T // NCH              # chunk width (4)
LO_S = np.float32(2.0 ** -11)
FP16_MIN_NORMAL = 2.0 ** -14

_CACHE = {}
LAST_RESULTS = None


def _split16(w):
    """fp32 array -> (hi fp16, lo fp16) with w ~= hi + 2^-11*lo, subnormals flushed."""
    hi = w.astype(np.float16)
    hi[np.abs(hi) < FP16_MIN_NORMAL] = 0
    lo = ((w - hi.astype(np.float32)) * np.float32(2.0 ** 11)).astype(np.float16)
    lo[np.abs(lo) < FP16_MIN_NORMAL] = 0
    return hi, lo


def _pack_blocks(cat, kt):
    """[kt*128, 384] -> [128 p, kt, NB, 128 col]"""
    return np.ascontiguousarray(cat.reshape(kt, 128, NB, 128).transpose(1, 0, 2, 3))


def _prep_weights(W_ih, b_ih, W_hh, b_hh, tau_m, tau_adp,
                  mask_ih, mask_hh, delay_ih, delay_hh):
    f32 = np.float32
    alpha = np.exp(-f32(DT) / tau_m).astype(f32)
    afac = (f32(1.0) - alpha) * f32(RM)          # folded into weights + bias
    rho = np.exp(-f32(DT) / tau_adp).astype(f32)

    Wm_ih = (np.abs(W_ih.T) * mask_ih.T).astype(f32) * afac[None, :]
    ei = np.concatenate([np.ones(N_EXC, f32), -np.ones(N_NEU - N_EXC, f32)])
    Wm_hh = (np.abs(W_hh.T) * mask_hh.T).astype(f32) * ei[:, None] * afac[None, :]

    wih_cat = np.zeros((KIH * 128, N_NEU), f32)
    for d in range(T_MAX):
        wih_cat[d * N_PRE:(d + 1) * N_PRE] = Wm_ih * (delay_ih == d)
    wih_cat[T_MAX * N_PRE] = (b_ih + b_hh).astype(f32) * afac

    whh_cat = np.zeros((KHH * 128, N_NEU), f32)
    for d in range(T_MAX):
        whh_cat[d * N_NEU:(d + 1) * N_NEU] = Wm_hh * (delay_hh == d)

    wih_hi, wih_lo = _split16(wih_cat)
    whh_hi, whh_lo = _split16(whh_cat)

    consts = np.zeros((128, 3, NB), f32)
    consts[:, 0, :] = alpha.reshape(NB, 128).T
    consts[:, 1, :] = rho.reshape(NB, 128).T
    consts[:, 2, :] = (f32(1.0) - rho).reshape(NB, 128).T
    return (_pack_blocks(wih_hi, KIH), _pack_blocks(wih_lo, KIH),
            _pack_blocks(whh_hi, KHH), _pack_blocks(whh_lo, KHH), consts)


def _prep_x(x):
    """x [T, B, N_PRE] -> per-core (x_hi, x_lo) [128, KIH, T] fp16."""
    f32 = np.float32
    out = []
    for b in range(B):
        xc = np.zeros((KIH * 128, T), f32)
        for d in range(T_MAX):
            for t in range(d, T):
                xc[d * N_PRE:(d + 1) * N_PRE, t] = x[t - d, b]
        xc[T_MAX * N_PRE, :] = 1.0  # bias row (exact in fp16; lo row -> 0)
        hi, lo = _split16(xc)
        out.append((np.ascontiguousarray(hi.reshape(KIH, 128, T).transpose(1, 0, 2)),
                    np.ascontiguousarray(lo.reshape(KIH, 128, T).transpose(1, 0, 2))))
    return out


def _hh_schedule():
    """Per slot t: list of (d, t0, w) hh batch instances emitted in slot t.

    Instance (d, t0, w) contributes to psum cols [t0, t0+w) using rhs spike
    cols [t0-1-d, t0-1-d+w).  Constraints: w <= d (so the freshest rhs col is
    s[t0-2], prefetchable in slot t0 before s[t0-1] lands), w <= chunk
    remainder (out cols in one bank), except d=0 which is w=1 rhs s[t0-1].
    """
    slots = {t: [] for t in range(T)}
    for d in range(T_MAX):
        t = d + 1  # first step with t-1-d >= 0
        while t < T:
            w = max(1, min(d, T - t, CW - (t % CW)))
            slots[t].append((d, t, w))
            t += w
    return slots


def _build():
    import concourse.bacc as bacc
    import concourse.tile as tile
    from concourse import mybir

    f32 = mybir.dt.float32
    f16 = mybir.dt.float16
    nc = bacc.Bacc("TRN2", target_bir_lowering=False, debug=False)
    xh_d = nc.dram_tensor("x_hi", [128, KIH, T], f16, kind="ExternalInput")
    xl_d = nc.dram_tensor("x_lo", [128, KIH, T], f16, kind="ExternalInput")
    wih_h_d = nc.dram_tensor("wih_hi", [128, KIH, NB, 128], f16, kind="ExternalInput")
    wih_l_d = nc.dram_tensor("wih_lo", [128, KIH, NB, 128], f16, kind="ExternalInput")
    whh_h_d = nc.dram_tensor("whh_hi", [128, KHH, NB, 128], f16, kind="ExternalInput")
    whh_l_d = nc.dram_tensor("whh_lo", [128, KHH, NB, 128], f16, kind="ExternalInput")
    cons_d = nc.dram_tensor("consts", [128, 3, NB], f32, kind="ExternalInput")
    sout_d = nc.dram_tensor("sout", [128, T, NB], f16, kind="ExternalOutput")

    slots = _hh_schedule()

    with tile.TileContext(nc) as tc:
        with (
            tc.tile_pool(name="singles", bufs=1) as singles,
            tc.tile_pool(name="state", bufs=3) as state,
            tc.tile_pool(name="tmp", bufs=3) as tmp,
            tc.tile_pool(name="psum", bufs=1, space="PSUM") as psp,
        ):
            xh_sb = singles.tile([128, KIH, T], f16)
            xl_sb = singles.tile([128, KIH, T], f16)
            wih_h = singles.tile([128, KIH, NB, 128], f16)
            wih_l = singles.tile([128, KIH, NB, 128], f16)
            whh_h = singles.tile([128, KHH, NB, 128], f16)
            whh_l = singles.tile([128, KHH, NB, 128], f16)
            cons_sb = singles.tile([128, 3, NB], f32)
            s16 = singles.tile([128, T, NB], f16)      # spikes (rhs + output)
            s16lo = singles.tile([128, T, NB], f16)    # spikes * 2^-11
            neg_b0 = singles.tile([128, NB], f32)

            # input DMAs, ordered by first use; weights split across queues
            nc.sync.dma_start(out=xh_sb[:], in_=xh_d[:])
            nc.sync.dma_start(out=xl_sb[:], in_=xl_d[:])
            nc.gpsimd.dma_start(out=cons_sb[:], in_=cons_d[:])
            nc.sync.dma_start(out=wih_h[:], in_=wih_h_d[:])
            nc.gpsimd.dma_start(out=wih_l[:], in_=wih_l_d[:])
            for d in range(T_MAX):
                sl = slice(d * NB, (d + 1) * NB)
                nc.sync.dma_start(out=whh_h[:, sl], in_=whh_h_d[:, sl])
                nc.gpsimd.dma_start(out=whh_l[:, sl], in_=whh_l_d[:, sl])
            nc.vector.memset(neg_b0[:], -float(B0))

            alpha = cons_sb[:, 0, :]
            rho = cons_sb[:, 1, :]
            rho1 = cons_sb[:, 2, :]

            # psum: 3 chunk tiles (4 step-columns each) + aux for ih lo terms
            pm = [psp.tile([128, NB, CW], f32, name=f"pm{c}", tag=f"pm{c}")
                  for c in range(NCH)]
            ps2 = psp.tile([128, NB, T], f32, name="ps2", tag="ps2")

            # ---- ih conv. T1 (hi*x_hi) initializes the chunk tiles;
            # T2 (hi*x_lo) + T3 (lo*x_hi) accumulate into ps2 (folded by 2^-11)
            # NOTE: start=True clears has_written for the WHOLE psum bank, so
            # exactly one start per bank (its first write); all later writes
            # use start=False (overwrite-if-unwritten, else accumulate).
            for c in range(NCH):
                for ob in range(NB):
                    for kt in range(KIH):
                        nc.tensor.matmul(
                            pm[c][:, ob, :], wih_h[:, kt, ob, :],
                            xh_sb[:, kt, c * CW:(c + 1) * CW],
                            start=(ob == 0 and kt == 0), stop=False)
            for ob in range(NB):
                for kt in range(KIH):
                    nc.tensor.matmul(ps2[:, ob, :], wih_h[:, kt, ob, :],
                                     xl_sb[:, kt, :],
                                     start=(ob == 0 and kt == 0), stop=False)
                for kt in range(KIH):
                    nc.tensor.matmul(ps2[:, ob, :], wih_l[:, kt, ob, :],
                                     xh_sb[:, kt, :], start=False,
                                     stop=(kt == KIH - 1))

            u_delta = None   # delta(t-1): u'' = delta + vthr*(delta<0)
            vthr_p = None    # vthr(t-1)
            eta_p = None     # eta(t-1)
            for t in range(T):
                c, j = divmod(t, CW)
                # ---- off-path state ops (depend on step t-1 results only)
                eta_t = state.tile([128, NB], f32, tag="eta")
                vthr = state.tile([128, NB], f32, tag="vthr")
                if eta_p is not None:
                    er = tmp.tile([128, NB], f32, tag="er")
                    sr = tmp.tile([128, NB], f32, tag="sr")
                    nc.vector.tensor_mul(er[:], eta_p[:], rho)
                    nc.vector.tensor_mul(sr[:], s16[:, t - 1, :], rho1)
                    nc.vector.tensor_add(eta_t[:], er[:], sr[:])
                    nc.vector.tensor_scalar(
                        out=vthr[:], in0=eta_t[:], scalar1=float(BETA),
                        scalar2=float(B0), op0=mybir.AluOpType.mult,
                        op1=mybir.AluOpType.add)
                else:
                    nc.vector.memset(eta_t[:], 0.0)
                    nc.vector.memset(vthr[:], float(B0))
                # wtmp = u''(t-1)*alpha - vthr(t);  wc = wtmp + 2^-11*ps2[:, :, t]
                wc = state.tile([128, NB], f32, tag="wc")
                if u_delta is not None:
                    sbar = tmp.tile([128, NB], f32, tag="sbar")
                    vs = tmp.tile([128, NB], f32, tag="vs")
                    u2 = tmp.tile([128, NB], f32, tag="u2")
                    ua = tmp.tile([128, NB], f32, tag="ua")
                    wt = tmp.tile([128, NB], f32, tag="wt")
                    nc.vector.tensor_scalar(out=sbar[:], in0=u_delta[:],
                                            scalar1=0.0, scalar2=None,
                                            op0=mybir.AluOpType.is_lt)
                    nc.vector.tensor_mul(vs[:], sbar[:], vthr_p[:])
                    nc.vector.tensor_add(u2[:], u_delta[:], vs[:])
                    nc.vector.tensor_mul(ua[:], u2[:], alpha)
                    nc.vector.tensor_sub(wt[:], ua[:], vthr[:])
                    nc.vector.scalar_tensor_tensor(
                        out=wc[:], in0=ps2[:, :, t], scalar=float(LO_S),
                        in1=wt[:], op0=mybir.AluOpType.mult,
                        op1=mybir.AluOpType.add)
                else:
                    nc.vector.scalar_tensor_tensor(
                        out=wc[:], in0=ps2[:, :, t], scalar=float(LO_S),
                        in1=neg_b0[:], op0=mybir.AluOpType.mult,
                        op1=mybir.AluOpType.add)

                # ---- hh matmuls for this slot (prefetchable first, d=0 last)
                for hi_pass in (True, False):
                    wsb = whh_h if hi_pass else whh_l
                    rsb = s16 if hi_pass else s16lo
                    for (d, t0, w) in slots[t]:
                        if d == 0:
                            continue
                        cc, jj = divmod(t0, CW)
                        for ob in range(NB):
                            for kt in range(NB):
                                nc.tensor.matmul(
                                    pm[cc][:, ob, jj:jj + w],
                                    wsb[:, d * NB + kt, ob, :],
                                    rsb[:, t0 - 1 - d:t0 - 1 - d + w, kt],
                                    start=False, stop=False)
                if t >= 1:
                    for hi_pass in (True, False):
                        wsb = whh_h if hi_pass else whh_l
                        rsb = s16 if hi_pass else s16lo
                        for ob in range(NB):
                            for kt in range(NB):
                                nc.tensor.matmul(
                                    pm[c][:, ob, j:j + 1],
                                    wsb[:, kt, ob, :],
                                    rsb[:, t - 1:t, kt],
                                    start=False,
                                    stop=(not hi_pass) and (kt == NB - 1))

                # ---- critical chain: delta = psum_col + wc ; s = (delta >= 0)
                delta = state.tile([128, NB], f32, tag="delta")
                nc.vector.scalar_tensor_tensor(
                    out=delta[:], in0=pm[c][:, :, j], scalar=1.0, in1=wc[:],
                    op0=mybir.AluOpType.mult, op1=mybir.AluOpType.add)
                nc.vector.tensor_scalar(
                    out=s16[:, t, :], in0=delta[:], scalar1=0.0, scalar2=None,
                    op0=mybir.AluOpType.is_ge)
                nc.vector.tensor_scalar(
                    out=s16lo[:, t, :], in0=delta[:], scalar1=0.0,
                    scalar2=float(LO_S), op0=mybir.AluOpType.is_ge,
                    op1=mybir.AluOpType.mult)

                u_delta, vthr_p, eta_p = delta, vthr, eta_t

            nc.sync.dma_start(out=sout_d[:], in_=s16[:])

    nc.compile()
    return nc


def kernel(x, W_ih, b_ih, W_hh, b_hh, tau_m, tau_adp,
           mask_ih, mask_hh, delay_ih, delay_hh, _trace=False, _tmpdir=None):
    global LAST_RESULTS
    from concourse.bass_utils import run_bass_kernel_spmd

    if "nc" not in _CACHE:
        _CACHE["nc"] = _build()
    nc = _CACHE["nc"]

    wih_hi, wih_lo, whh_hi, whh_lo, consts = _prep_weights(
        np.asarray(W_ih), np.asarray(b_ih), np.asarray(W_hh), np.asarray(b_hh),
        np.asarray(tau_m), np.asarray(tau_adp), np.asarray(mask_ih),
        np.asarray(mask_hh), np.asarray(delay_ih), np.asarray(delay_hh))
    xs = _prep_x(np.asarray(x))

    in_maps = [
        {"x_hi": xs[b][0], "x_lo": xs[b][1], "wih_hi": wih_hi,
         "wih_lo": wih_lo, "whh_hi": whh_hi, "whh_lo": whh_lo,
         "consts": consts}
        for b in range(B)
    ]
    res = run_bass_kernel_spmd(nc, in_maps, list(range(B)),
                               trace=_trace, tmpdir=_tmpdir)
    LAST_RESULTS = res
    out = np.empty((T, B, N_NEU), np.float32)
    for b in range(B):
        sb = res.results[b]["sout"]          # [128, T, NB] fp16
        out[:, b, :] = sb.astype(np.float32).transpose(1, 2, 0).reshape(T, N_NEU)
    return out


# revision 6
# speedup vs baseline: 3.8854x; 1.0398x over previous
"""Trainium2 Bass kernel for nn_ExcitatoryInhibitoryBlock.

Math reformulation (eliminates the [T_MAX, B, in, out] scatter buffers):
  o_ih[t] = b_ih + sum_d x[t-d] @ Wd_ih[d],  Wd_ih[d][i,o] = |W_ih.T|*mask_ih.T * (delay_ih==d)
  o_hh[t] = b_hh + sum_d (s[t-1-d]*ei) @ Wd_hh[d]
  u  = u*alpha + (1-alpha)*RM*(o_ih + o_hh)
  s  = (u - (B0 + BETA*eta) >= 0);  u -= s*(BETA*eta + B0);  eta = rho*eta + (1-rho)*s

Sharding: data-parallel over batch B=8 -> one batch element per NeuronCore,
no collectives (the recurrence is independent per batch element).

Precision scheme: the PE runs fp16 (fp32 matmul is ~4x slower / 2-pass).
(1-alpha)*RM is folded into all synapse weights + bias on the host, then each
folded weight W' is split W' ~= W_hi + 2^-11 * W_lo with W_hi = fp16(W'),
W_lo = fp16((W' - W_hi) * 2^11).  The 2^-11 rides on the rhs: spikes are fed
twice (s and s*2^-11, both exact in fp16), x is split the same way.  Residual
error is ~2^-22 * |W'| with |W'| ~ 5e-3 * |W|, far below fp32 rounding of u.

Device layout: neurons n = ob*128 + p (p = SBUF partition, ob in 0..2).
Weights are host-prepacked [128 p, ktile, ob, col]; the tensor engine runs
out[128 o, N] += W_block[128 i, 128 o]^T @ rhs[128 i, N].

Schedule: PSUM holds i_t*afac for 4 steps per bank (3 chunk tiles).  The ih
conv (T1) initializes them; recurrent blocks accumulate start=False.  Delay-d
contributions for steps [t0, t0+w) batch into one matmul (w <= d, rhs =
consecutive spike columns) emitted in slot t0 so only d=0 blocks sit on the
s[t-1] -> s[t] critical path.  Off-path state ops (eta, vthr, u'', wc) run
while the PE does next-slot prefetch; the critical DVE chain per step is
delta = psum_col + wc ; s = is_ge(delta, 0).
"""

import sys

if "/opt/trn_rl_repo" not in sys.path:
    sys.path.insert(0, "/opt/trn_rl_repo")

import numpy as np

T, T_MAX = 12, 4
B = 8
N_PRE, N_NEU = 192, 384
N_EXC = int(N_NEU * 0.8)
RM, DT, B0, BETA = 1.0, 1.0, 0.01, 1.8

NB = N_NEU // 128          # 3 output column-blocks
KIH = 7                    # ceil((T_MAX*N_PRE + 1)/128): 768 rows + bias row
KHH = T_MAX * NB           # 12 k-tiles of delay-expanded hh weights
NCH = 3                    # psum chunks of 4 steps
CW = T // NCH              # chunk width (4)
LO_S = np.float32(2.0 ** -11)
FP16_MIN_NORMAL = 2.0 ** -14

_CACHE = {}
LAST_RESULTS = None


def _split16(w):
    """fp32 array -> (hi fp16, lo fp16) with w ~= hi + 2^-11*lo, subnormals flushed."""
    hi = w.astype(np.float16)
    hi[np.abs(hi) < FP16_MIN_NORMAL] = 0
    lo = ((w - hi.astype(np.float32)) * np.float32(2.0 ** 11)).astype(np.float16)
    lo[np.abs(lo) < FP16_MIN_NORMAL] = 0
    return hi, lo


def _pack_blocks(cat, kt):
    """[kt*128, 384] -> [128 p, kt, NB, 128 col]"""
    return np.ascontiguousarray(cat.reshape(kt, 128, NB, 128).transpose(1, 0, 2, 3))


def _prep_weights(W_ih, b_ih, W_hh, b_hh, tau_m, tau_adp,
                  mask_ih, mask_hh, delay_ih, delay_hh):
    f32 = np.float32
    alpha = np.exp(-f32(DT) / tau_m).astype(f32)
    afac = (f32(1.0) - alpha) * f32(RM)          # folded into weights + bias
    rho = np.exp(-f32(DT) / tau_adp).astype(f32)

    Wm_ih = (np.abs(W_ih.T) * mask_ih.T).astype(f32) * afac[None, :]
    ei = np.concatenate([np.ones(N_EXC, f32), -np.ones(N_NEU - N_EXC, f32)])
    Wm_hh = (np.abs(W_hh.T) * mask_hh.T).astype(f32) * ei[:, None] * afac[None, :]

    wih_cat = np.zeros((KIH * 128, N_NEU), f32)
    for d in range(T_MAX):
        wih_cat[d * N_PRE:(d + 1) * N_PRE] = Wm_ih * (delay_ih == d)
    wih_cat[T_MAX * N_PRE] = (b_ih + b_hh).astype(f32) * afac

    whh_cat = np.zeros((KHH * 128, N_NEU), f32)
    for d in range(T_MAX):
        whh_cat[d * N_NEU:(d + 1) * N_NEU] = Wm_hh * (delay_hh == d)

    wih_hi, wih_lo = _split16(wih_cat)
    whh_hi, whh_lo = _split16(whh_cat)

    consts = np.zeros((128, 3, NB), f32)
    consts[:, 0, :] = alpha.reshape(NB, 128).T
    consts[:, 1, :] = rho.reshape(NB, 128).T
    consts[:, 2, :] = (f32(1.0) - rho).reshape(NB, 128).T
    return (_pack_blocks(wih_hi, KIH), _pack_blocks(wih_lo, KIH),
            _pack_blocks(whh_hi, KHH), _pack_blocks(whh_lo, KHH), consts)


def _prep_x(x):
    """x [T, B, N_PRE] -> per-core (x_hi, x_lo) [128, KIH, T] fp16."""
    f32 = np.float32
    out = []
    for b in range(B):
        xc = np.zeros((KIH * 128, T), f32)
        for d in range(T_MAX):
            for t in range(d, T):
                xc[d * N_PRE:(d + 1) * N_PRE, t] = x[t - d, b]
        xc[T_MAX * N_PRE, :] = 1.0  # bias row (exact in fp16; lo row -> 0)
        hi, lo = _split16(xc)
        out.append((np.ascontiguousarray(hi.reshape(KIH, 128, T).transpose(1, 0, 2)),
                    np.ascontiguousarray(lo.reshape(KIH, 128, T).transpose(1, 0, 2))))
    return out


def _hh_schedule():
    """Per slot t: list of (d, t0, w) hh batch instances emitted in slot t.

    Instance (d, t0, w) contributes to psum cols [t0, t0+w) using rhs spike
    cols [t0-1-d, t0-1-d+w).  Constraints: w <= d (so the freshest rhs col is
    s[t0-2], prefetchable in slot t0 before s[t0-1] lands), w <= chunk
    remainder (out cols in one bank), except d=0 which is w=1 rhs s[t0-1].
    """
    slots = {t: [] for t in range(T)}
    for d in range(T_MAX):
        t = d + 1  # first step with t-1-d >= 0
        while t < T:
            w = max(1, min(d, T - t, CW - (t % CW)))
            slots[t].append((d, t, w))
            t += w
    return slots


def _build():
    import concourse.bacc as bacc
    import concourse.tile as tile
    from concourse import mybir

    f32 = mybir.dt.float32
    f16 = mybir.dt.float16
    nc = bacc.Bacc("TRN2", target_bir_lowering=False, debug=False)
    xh_d = nc.dram_tensor("x_hi", [128, KIH, T], f16, kind="ExternalInput")
    xl_d = nc.dram_tensor("x_lo", [128, KIH, T], f16, kind="ExternalInput")
    wih_h_d = nc.dram_tensor("wih_hi", [128, KIH, NB, 128], f16, kind="ExternalInput")
    wih_l_d = nc.dram_tensor("wih_lo", [128, KIH, NB, 128], f16, kind="ExternalInput")
    whh_h_d = nc.dram_tensor("whh_hi", [128, KHH, NB, 128], f16, kind="ExternalInput")
    whh_l_d = nc.dram_tensor("whh_lo", [128, KHH, NB, 128], f16, kind="ExternalInput")
    cons_d = nc.dram_tensor("consts", [128, 3, NB], f32, kind="ExternalInput")
    sout_d = nc.dram_tensor("sout", [128, T, NB], f16, kind="ExternalOutput")

    slots = _hh_schedule()

    with tile.TileContext(nc) as tc:
        with (
            tc.tile_pool(name="singles", bufs=1) as singles,
            tc.tile_pool(name="state", bufs=3) as state,
            tc.tile_pool(name="tmp", bufs=3) as tmp,
            tc.tile_pool(name="psum", bufs=1, space="PSUM") as psp,
        ):
            xh_sb = singles.tile([128, KIH, T], f16)
            xl_sb = singles.tile([128, KIH, T], f16)
            wih_h = singles.tile([128, KIH, NB, 128], f16)
            wih_l = singles.tile([128, KIH, NB, 128], f16)
            whh_h = singles.tile([128, KHH, NB, 128], f16)
            whh_l = singles.tile([128, KHH, NB, 128], f16)
            cons_sb = singles.tile([128, 3, NB], f32)
            s16 = singles.tile([128, T, NB], f16)      # spikes (rhs + output)
            s16lo = singles.tile([128, T, NB], f16)    # spikes * 2^-11
            neg_b0 = singles.tile([128, NB], f32)

            # input DMAs, ordered by first use, on two otherwise-idle engine
            # queues (sync + scalar); gpsimd runs Tile bookkeeping and issues
            # DMA descriptors too slowly (measured ~3 GB/s vs ~12 GB/s).
            nc.sync.dma_start(out=xh_sb[:], in_=xh_d[:])
            nc.scalar.dma_start(out=xl_sb[:], in_=xl_d[:])
            nc.scalar.dma_start(out=cons_sb[:], in_=cons_d[:])
            nc.sync.dma_start(out=wih_h[:], in_=wih_h_d[:])
            nc.scalar.dma_start(out=wih_l[:], in_=wih_l_d[:])
            for d in range(T_MAX):
                sl = slice(d * NB, (d + 1) * NB)
                nc.sync.dma_start(out=whh_h[:, sl], in_=whh_h_d[:, sl])
                nc.scalar.dma_start(out=whh_l[:, sl], in_=whh_l_d[:, sl])
            nc.vector.memset(neg_b0[:], -float(B0))

            alpha = cons_sb[:, 0, :]
            rho = cons_sb[:, 1, :]
            rho1 = cons_sb[:, 2, :]

            # psum: 3 chunk tiles (4 step-columns each) + aux for ih lo terms
            pm = [psp.tile([128, NB, CW], f32, name=f"pm{c}", tag=f"pm{c}")
                  for c in range(NCH)]
            ps2 = psp.tile([128, NB, T], f32, name="ps2", tag="ps2")

            # ---- ih conv. T1 (hi*x_hi) initializes the chunk tiles;
            # T2 (hi*x_lo) + T3 (lo*x_hi) accumulate into ps2 (folded by 2^-11)
            # NOTE: start=True clears has_written for the WHOLE psum bank, so
            # exactly one start per bank (its first write); all later writes
            # use start=False (overwrite-if-unwritten, else accumulate).
            for c in range(NCH):
                for ob in range(NB):
                    for kt in range(KIH):
                        nc.tensor.matmul(
                            pm[c][:, ob, :], wih_h[:, kt, ob, :],
                            xh_sb[:, kt, c * CW:(c + 1) * CW],
                            start=(ob == 0 and kt == 0), stop=False)
            for ob in range(NB):
                for kt in range(KIH):
                    nc.tensor.matmul(ps2[:, ob, :], wih_h[:, kt, ob, :],
                                     xl_sb[:, kt, :],
                                     start=(ob == 0 and kt == 0), stop=False)
                for kt in range(KIH):
                    nc.tensor.matmul(ps2[:, ob, :], wih_l[:, kt, ob, :],
                                     xh_sb[:, kt, :], start=False,
                                     stop=(kt == KIH - 1))

            u_delta = None   # delta(t-1): u'' = delta + vthr*(delta<0)
            vthr_p = None    # vthr(t-1)
            eta_p = None     # eta(t-1)
            for t in range(T):
                c, j = divmod(t, CW)
                # ---- off-path state ops (depend on step t-1 results only)
                eta_t = state.tile([128, NB], f32, tag="eta")
                vthr = state.tile([128, NB], f32, tag="vthr")
                if eta_p is not None:
                    er = tmp.tile([128, NB], f32, tag="er")
                    sr = tmp.tile([128, NB], f32, tag="sr")
                    nc.vector.tensor_mul(er[:], eta_p[:], rho)
                    nc.vector.tensor_mul(sr[:], s16[:, t - 1, :], rho1)
                    nc.vector.tensor_add(eta_t[:], er[:], sr[:])
                    nc.vector.tensor_scalar(
                        out=vthr[:], in0=eta_t[:], scalar1=float(BETA),
                        scalar2=float(B0), op0=mybir.AluOpType.mult,
                        op1=mybir.AluOpType.add)
                else:
                    nc.vector.memset(eta_t[:], 0.0)
                    nc.vector.memset(vthr[:], float(B0))
                # wtmp = u''(t-1)*alpha - vthr(t);  wc = wtmp + 2^-11*ps2[:, :, t]
                wc = state.tile([128, NB], f32, tag="wc")
                if u_delta is not None:
                    sbar = tmp.tile([128, NB], f32, tag="sbar")
                    vs = tmp.tile([128, NB], f32, tag="vs")
                    u2 = tmp.tile([128, NB], f32, tag="u2")
                    ua = tmp.tile([128, NB], f32, tag="ua")
                    wt = tmp.tile([128, NB], f32, tag="wt")
                    nc.vector.tensor_scalar(out=sbar[:], in0=u_delta[:],
                                            scalar1=0.0, scalar2=None,
                                            op0=mybir.AluOpType.is_lt)
                    nc.vector.tensor_mul(vs[:], sbar[:], vthr_p[:])
                    nc.vector.tensor_add(u2[:], u_delta[:], vs[:])
                    nc.vector.tensor_mul(ua[:], u2[:], alpha)
                    nc.vector.tensor_sub(wt[:], ua[:], vthr[:])
                    nc.vector.scalar_tensor_tensor(
                        out=wc[:], in0=ps2[:, :, t], scalar=float(LO_S),
                        in1=wt[:], op0=mybir.AluOpType.mult,
                        op1=mybir.AluOpType.add)
                else:
                    nc.vector.scalar_tensor_tensor(
                        out=wc[:], in0=ps2[:, :, t], scalar=float(LO_S),
                        in1=neg_b0[:], op0=mybir.AluOpType.mult,
                        op1=mybir.AluOpType.add)

                # ---- hh matmuls for this slot (prefetchable first, d=0 last)
                for hi_pass in (True, False):
                    wsb = whh_h if hi_pass else whh_l
                    rsb = s16 if hi_pass else s16lo
                    for (d, t0, w) in slots[t]:
                        if d == 0:
                            continue
                        cc, jj = divmod(t0, CW)
                        for ob in range(NB):
                            for kt in range(NB):
                                nc.tensor.matmul(
                                    pm[cc][:, ob, jj:jj + w],
                                    wsb[:, d * NB + kt, ob, :],
                                    rsb[:, t0 - 1 - d:t0 - 1 - d + w, kt],
                                    start=False, stop=False)
                if t >= 1:
                    for hi_pass in (True, False):
                        wsb = whh_h if hi_pass else whh_l
                        rsb = s16 if hi_pass else s16lo
                        for ob in range(NB):
                            for kt in range(NB):
                                nc.tensor.matmul(
                                    pm[c][:, ob, j:j + 1],
                                    wsb[:, kt, ob, :],
                                    rsb[:, t - 1:t, kt],
                                    start=False,
                                    stop=(not hi_pass) and (kt == NB - 1))

                # ---- critical chain: delta = psum_col + wc ; s = (delta >= 0)
                delta = state.tile([128, NB], f32, tag="delta")
                nc.vector.scalar_tensor_tensor(
                    out=delta[:], in0=pm[c][:, :, j], scalar=1.0, in1=wc[:],
                    op0=mybir.AluOpType.mult, op1=mybir.AluOpType.add)
                nc.vector.tensor_scalar(
                    out=s16[:, t, :], in0=delta[:], scalar1=0.0, scalar2=None,
                    op0=mybir.AluOpType.is_ge)
                nc.vector.tensor_scalar(
                    out=s16lo[:, t, :], in0=delta[:], scalar1=0.0,
                    scalar2=float(LO_S), op0=mybir.AluOpType.is_ge,
                    op1=mybir.AluOpType.mult)

                u_delta, vthr_p, eta_p = delta, vthr, eta_t

            nc.sync.dma_start(out=sout_d[:], in_=s16[:])

    nc.compile()
    return nc


def kernel(x, W_ih, b_ih, W_hh, b_hh, tau_m, tau_adp,
           mask_ih, mask_hh, delay_ih, delay_hh, _trace=False, _tmpdir=None):
    global LAST_RESULTS
    from concourse.bass_utils import run_bass_kernel_spmd

    if "nc" not in _CACHE:
        _CACHE["nc"] = _build()
    nc = _CACHE["nc"]

    wih_hi, wih_lo, whh_hi, whh_lo, consts = _prep_weights(
        np.asarray(W_ih), np.asarray(b_ih), np.asarray(W_hh), np.asarray(b_hh),
        np.asarray(tau_m), np.asarray(tau_adp), np.asarray(mask_ih),
        np.asarray(mask_hh), np.asarray(delay_ih), np.asarray(delay_hh))
    xs = _prep_x(np.asarray(x))

    in_maps = [
        {"x_hi": xs[b][0], "x_lo": xs[b][1], "wih_hi": wih_hi,
         "wih_lo": wih_lo, "whh_hi": whh_hi, "whh_lo": whh_lo,
         "consts": consts}
        for b in range(B)
    ]
    res = run_bass_kernel_spmd(nc, in_maps, list(range(B)),
                               trace=_trace, tmpdir=_tmpdir)
    LAST_RESULTS = res
    out = np.empty((T, B, N_NEU), np.float32)
    for b in range(B):
        sb = res.results[b]["sout"]          # [128, T, NB] fp16
        out[:, b, :] = sb.astype(np.float32).transpose(1, 2, 0).reshape(T, N_NEU)
    return out
